# revision 42
# baseline (speedup 1.0000x reference)
"""Masked self-attention Trainium2 kernel (8 NeuronCores, Bass/Tile).

Problem: B=4, S=2048, D=1024, DK=128 fp32.
  Q = X@Wq + bq; K = X@Wk + bk; V = X@Wv + bv
  scores = Q@K^T / sqrt(DK); masked = scores + tril(ones)*(-1e9)
  out = softmax(masked) @ V

Sharding: core = (batch b = core//2) x (row-half h = core%2). Each core
computes 64 query rows of each of the 16 query tiles of its batch
(rows 128c + 64h + j). All cores run an identical program; per-core
differences are carried entirely in the input data (a column
permutation of X^T and a small mask block).

Device layouts (all transposed so the PE contracts over partitions):
  X^T [D, S] (host-transposed, per-tile column permuted: own rows
  first), streamed in 2 superblocks of 1024 columns.
  Q^T/K^T/V^T [DK, *] = W-chunks(lhsT) x X^T(moving) fp16 matmuls with
  fp32 PSUM accumulation; Q projected only for the core's own 64-column
  tile halves (strided moving AP), 512 columns per superblock.
  scores^T [s-chunk 128, q-cols] = K^T-chunk(lhsT) x Q^T(moving)
  causal skip: chunk c only attends query tiles qi <= c -> contiguous
  q-prefix of width 64*(c+1); single [128,64] mask block on the
  diagonal tile. The -1e9 add absorbs the score entirely in fp32
  (|score| << ulp(1e9)), matching the reference bit pattern, and exp
  underflows masked lanes to exactly 0.
  softmax: exp without max-subtraction (scores are O(1)); row sums
  accumulated on the DVE into an fp16 staging tile (sacc += pt per
  piece) and reduced once per 512-column half with an M=1 all-ones
  matmul -- this removes a third matmul pass over every attention
  piece. Normalization via DVE reciprocal_approx_fast (no Ln/Exp
  activation-table reloads; the scalar engine only ever loads the Exp
  table) and a K=1 matmul broadcast of 1/sums across partitions.
  out^T [DK, 1024] accumulated in PSUM across s-chunks.

  Schedule: superblock-0 projections; then the attention pieces that
  need only chunks 7..1 (filling the PE while superblock 1 streams
  from HBM); superblock-1 projections; phase A (query columns
  [512,1024), chunks 15..8); phase B (columns [0,512), chunks 15..8);
  the chunk-0 piece last. The upper output half is summed/normalized/
  DMA'd while phase B still streams on the PE, so only the lower
  half's normalize chain sits on the critical tail. The globally
  fully-masked last row (2047) is patched on the host with mean(V) --
  exactly what the fp32 reference computes for it, since scores - 1e9
  == -1e9 in fp32 makes its softmax uniform. Its on-device column
  underflows to 0/0 = NaN and is overwritten.

  All matmul operands are float16 (11-bit mantissa, ~2.4e-4 rounding)
  with fp32 PSUM accumulation; the output is stored fp16 (halves the
  store DMA) and widened on the host. The first weight chunk gets a
  dedicated small first-wave DMA because the DGE queues fair-share HBM
  bandwidth and gate the first matmul.
"""

import numpy as np

import concourse.bacc as bacc
import concourse.tile as tile
import concourse.mybir as mybir
from concourse.bass_utils import run_bass_kernel_spmd

F32 = mybir.dt.float32
F16 = mybir.dt.float16
AF = mybir.ActivationFunctionType

B, S, D, DK = 4, 2048, 1024, 128
NEG = -1.0e9
NCORES = 8
NSB = 2           # superblocks of 1024 s-columns
NCHUNK = 16       # s-chunks of 128
QL = 1024         # local query columns per core (16 tiles x 64)

_cache = {}


def _build():
    nc = bacc.Bacc("TRN2", target_bir_lowering=False, debug=False,
                   num_devices=NCORES)

    xt = nc.dram_tensor("xt", [D, S], F16, kind="ExternalInput")
    wq = nc.dram_tensor("wq", [128, 8, DK], F16, kind="ExternalInput")
    wk = nc.dram_tensor("wk", [128, 8, DK], F16, kind="ExternalInput")
    wv = nc.dram_tensor("wv", [128, 8, DK], F16, kind="ExternalInput")
    bq = nc.dram_tensor("bq", [DK, 1], F32, kind="ExternalInput")
    bk = nc.dram_tensor("bk", [DK, 1], F32, kind="ExternalInput")
    bv = nc.dram_tensor("bv", [DK, 1], F32, kind="ExternalInput")
    maskd = nc.dram_tensor("maskd", [128, 64], F32, kind="ExternalInput")
    onesd = nc.dram_tensor("onesd", [128, 1], F16, kind="ExternalInput")
    idend = nc.dram_tensor("idend", [128, 128], F16, kind="ExternalInput")
    onesfd = nc.dram_tensor("onesfd", [1, 128], F16, kind="ExternalInput")
    outT = nc.dram_tensor("outT", [DK, QL], F16, kind="ExternalOutput")

    with tile.TileContext(nc) as tc:
        with (
            tc.tile_pool(name="consts", bufs=1) as cpool,
            tc.tile_pool(name="xblk", bufs=2) as xpool,
            tc.tile_pool(name="kv", bufs=1) as kvpool,
            tc.tile_pool(name="pt", bufs=5) as ppool,
            tc.tile_pool(name="outp", bufs=1) as opool,
            tc.tile_pool(name="ps_out", bufs=1, space="PSUM") as ps_out_pool,
            tc.tile_pool(name="ps_proj", bufs=2, space="PSUM") as ps_proj_pool,
            tc.tile_pool(name="ps_score", bufs=4, space="PSUM") as ps_score_pool,
        ):
            # ---- weights first (needed by the very first matmul).
            # wq's DMA is emitted after superblock 0's X so the K/V path
            # gets the HBM bandwidth first.
            w_sb = {}
            for name, dram in (("k", wk), ("v", wv), ("q", wq)):
                t = cpool.tile([128, 8, DK], F16, tag=f"w{name}")
                if name == "k":
                    nc.scalar.dma_start(out=t[:, 0:1], in_=dram[:, 0:1])
                    nc.scalar.dma_start(out=t[:, 1:8], in_=dram[:, 1:8])
                elif name == "v":
                    nc.scalar.dma_start(out=t[:], in_=dram[:])
                w_sb[name] = t

            def small_consts():
                b_sb = {}
                for name, dram in (("q", bq), ("k", bk), ("v", bv)):
                    t = cpool.tile([DK, 1], F32, tag=f"b{name}")
                    nc.gpsimd.dma_start(out=t[:], in_=dram[:])
                    b_sb[name] = t
                mask_sb = cpool.tile([128, 64], F32, tag="mask")
                nc.gpsimd.dma_start(out=mask_sb[:], in_=maskd[:])
                ones_sb = cpool.tile([128, 1], F16, tag="ones")
                nc.gpsimd.dma_start(out=ones_sb[:], in_=onesd[:])
                iden_sb = cpool.tile([128, 128], F16, tag="iden")
                nc.gpsimd.dma_start(out=iden_sb[:], in_=idend[:])
                onesf_sb = cpool.tile([1, 128], F16, tag="onesf")
                nc.gpsimd.dma_start(out=onesf_sb[:], in_=onesfd[:])
                return b_sb, mask_sb, ones_sb, iden_sb, onesf_sb

            # ---- persistent buffers ----
            kT_sb = kvpool.tile([DK, S], F16, tag="kT")
            vT_sb = kvpool.tile([DK, S], F16, tag="vT")
            qT_sb = kvpool.tile([DK, QL], F16, tag="qT")
            vnat_sb = kvpool.tile([128, NCHUNK, DK], F16, tag="vnat")
            sacc_sb = kvpool.tile([128, QL], F16, tag="sacc")
            o_sb = opool.tile([DK, QL], F16, tag="o")
            recip_u = opool.tile([1, 512], F32, tag="recip_u")
            recip_l = opool.tile([1, 512], F32, tag="recip_l")
            recip_u16 = opool.tile([1, 512], F16, tag="recip_u16")
            recip_l16 = opool.tile([1, 512], F16, tag="recip_l16")
            rbu_sb = opool.tile([128, 512], F32, tag="rbu")
            rbl_sb = opool.tile([128, 512], F32, tag="rbl")

            ps_out = ps_out_pool.tile([DK, QL], F32)       # 2 banks
            nc.vector.memset(ps_out[:], 0.0)
            nc.vector.memset(sacc_sb[:], 0.0)

            # ---- projections for one superblock of 1024 columns ----
            def proj_superblock(sb, b_sb, iden_sb):
                s0 = 1024 * sb
                xb = xpool.tile([128, 8, 1024], F16, tag="xb")
                # sb0 on the sync queue, sb1 on the scalar queue so both
                # issue early and share HBM fairly.
                eng = nc.sync
                # Fine-grained pieces let the dc-accumulation matmuls start
                # on partial data instead of stalling for a full 1 MiB half.
                for (d0, d1, half) in ((0, 2, 0), (2, 5, 0), (5, 8, 0),
                                       (0, 4, 1), (4, 8, 1)):
                    eng.dma_start(
                        out=xb[:, d0:d1, 512 * half:512 * half + 512],
                        in_=xt[128 * d0:128 * d1,
                               s0 + 512 * half:s0 + 512 * half + 512]
                        .rearrange("(i p) s -> p i s", p=128))
                if sb == 0:
                    # wq (not needed until Q proj ~19us in) goes on the
                    # gpsimd queue, out of the early HBM rush
                    nc.gpsimd.dma_start(out=w_sb["q"][:], in_=wq[:])

                for half in range(2):
                    cl = slice(s0 + 512 * half, s0 + 512 * half + 512)
                    xl = slice(512 * half, 512 * half + 512)
                    for name, dst in (("k", kT_sb), ("v", vT_sb)):
                        pp = ps_proj_pool.tile([DK, 512], F32, tag="pp")
                        for dc in range(8):
                            nc.tensor.matmul(
                                pp[:], w_sb[name][:, dc], xb[:, dc, xl],
                                start=(dc == 0), stop=(dc == 7))
                        nc.scalar.activation(dst[:, cl], pp[:],
                                             AF.Identity,
                                             bias=b_sb[name][:])

                # Q: first 64 cols of each 128-tile (own queries)
                pq = ps_proj_pool.tile([DK, 512], F32, tag="pp")
                for dc in range(8):
                    qmov = (xb[:, dc].rearrange("p (t j) -> p t j", t=8)
                            [:, :, 0:64])
                    nc.tensor.matmul(pq[:], w_sb["q"][:, dc], qmov,
                                     start=(dc == 0), stop=(dc == 7))
                q0 = 512 * sb
                nc.scalar.activation(qT_sb[:, q0:q0 + 512], pq[:],
                                     AF.Identity, bias=b_sb["q"][:])

                # V natural tiles (transpose V^T chunks)
                for g in range(2):
                    tp = ps_proj_pool.tile([128, 4, 128], F16, tag="pp")
                    for t in range(4):
                        c = 8 * sb + 4 * g + t
                        nc.tensor.matmul(
                            tp[:, t], vT_sb[:, 128 * c:128 * c + 128],
                            iden_sb[:], is_transpose=True,
                            start=(t == 0), stop=(t == 3))
                    c0 = 8 * sb + 4 * g
                    nc.vector.tensor_copy(vnat_sb[:, c0:c0 + 4], tp[:])

            # ---- attention pieces, software-pipelined by one stage:
            # piece i's PV matmul is emitted after piece i+1's score+exp,
            # so the PE always has a score to run while an exp is pending
            pending = []

            def _flush_one():
                c, p0, pn, pt, stop = pending.pop(0)
                nc.tensor.matmul(ps_out[:, p0:p0 + pn], vnat_sb[:, c],
                                 pt[:, 0:pn], start=False, stop=stop)
                nc.vector.tensor_tensor(
                    sacc_sb[:, p0:p0 + pn], sacc_sb[:, p0:p0 + pn],
                    pt[:, 0:pn], mybir.AluOpType.add)

            def flush_all():
                while pending:
                    _flush_one()

            def attn_piece(c, p0, pn, stop=False):
                kT_c = kT_sb[:, 128 * c:128 * c + 128]
                sc = ps_score_pool.tile([128, 512], F32, tag="sc")
                nc.tensor.matmul(sc[:, 0:pn], kT_c, qT_sb[:, p0:p0 + pn],
                                 start=True, stop=True)
                dcol = 64 * c
                if p0 <= dcol < p0 + pn:
                    dl = dcol - p0
                    nc.vector.tensor_tensor(
                        sc[:, dl:dl + 64], sc[:, dl:dl + 64],
                        mask_sb[:], mybir.AluOpType.add)
                pt = ppool.tile([128, 512], F16, tag="pt")
                nc.scalar.activation(pt[:, 0:pn], sc[:, 0:pn], AF.Exp)
                while len(pending) >= 2:
                    _flush_one()
                pending.append((c, p0, pn, pt, stop))

            # --- schedule ---
            # superblock 0 projections
            # (consts are DMA'd on the gpsimd queue concurrently)
            (b_sb, mask_sb, ones_sb, iden_sb, onesf_sb) = small_consts()
            proj_superblock(0, b_sb, iden_sb)

            # early pieces: chunks 7..1 (chunk 0 is saved for the very end)
            for c in range(7, 0, -1):
                attn_piece(c, 0, 64 * (c + 1))

            # superblock 1 projections
            proj_superblock(1, b_sb, iden_sb)

            # Phase A: query columns [512, 1024) -- chunks 15..8
            for c in range(15, 7, -1):
                attn_piece(c, 512, 64 * (c + 1) - 512, stop=(c == 8))

            # upper half row sums (waits on phase A's sacc adds)
            flush_all()
            sums_u = ps_score_pool.tile([1, 512], F32, tag="sc")
            nc.tensor.matmul(sums_u[:], ones_sb[:, 0:1],
                             sacc_sb[:, 512:1024], start=True, stop=True)

            # Phase B: chunks 15..8, columns [0, 512)
            for c in range(15, 7, -1):
                attn_piece(c, 0, 512)
                if c == 12:
                    # upper finalize, overlapping the remaining pieces
                    nc.vector.reciprocal_approx_fast(recip_u[:], sums_u[:])
                    nc.vector.tensor_copy(recip_u16[:], recip_u[:])
                    rb_u = ps_score_pool.tile([128, 512], F32, tag="sc")
                    nc.tensor.matmul(rb_u[:], onesf_sb[:], recip_u16[:],
                                     start=True, stop=True)
                if c == 10:
                    nc.vector.tensor_copy(rbu_sb[:], rb_u[:])
                    nc.vector.tensor_tensor(
                        o_sb[:, 512:1024], ps_out[:, 512:1024],
                        rbu_sb[:], mybir.AluOpType.mult)
                    nc.gpsimd.dma_start(out=outT[:, 512:1024],
                                        in_=o_sb[:, 512:1024])

            # final tiny piece: chunk 0, columns [0, 64); flush the
            # pending PV/sum work first so only chunk 0's remains at the end
            flush_all()
            attn_piece(0, 0, 64, stop=True)

            # lower finalize
            flush_all()
            sums_l = ps_score_pool.tile([1, 512], F32, tag="sc")
            nc.tensor.matmul(sums_l[:], ones_sb[:, 0:1], sacc_sb[:, 0:512],
                             start=True, stop=True)
            nc.vector.reciprocal_approx_fast(recip_l[:], sums_l[:])
            nc.vector.tensor_copy(recip_l16[:], recip_l[:])
            rb_l = ps_score_pool.tile([128, 512], F32, tag="sc")
            nc.tensor.matmul(rb_l[:], onesf_sb[:], recip_l16[:],
                             start=True, stop=True)
            nc.vector.tensor_copy(rbl_sb[:], rb_l[:])
            nc.vector.tensor_tensor(o_sb[:, 0:512], ps_out[:, 0:512],
                                    rbl_sb[:], mybir.AluOpType.mult)
            nc.sync.dma_start(out=outT[:, 0:512], in_=o_sb[:, 0:512])

    nc.compile()
    return nc


def _prep_inputs(inputs, Wq, bq, Wk, bk, Wv, bv):
    scale = np.float32(1.0 / np.sqrt(DK))
    wq_s = np.ascontiguousarray((Wq * scale).reshape(8, 128, DK).transpose(1, 0, 2)).astype(np.float16)
    wk_s = np.ascontiguousarray(Wk.reshape(8, 128, DK).transpose(1, 0, 2)).astype(np.float16)
    wv_s = np.ascontiguousarray(Wv.reshape(8, 128, DK).transpose(1, 0, 2)).astype(np.float16)
    bq_s = np.ascontiguousarray((bq * scale).reshape(DK, 1), dtype=np.float32)
    bk_s = np.ascontiguousarray(bk.reshape(DK, 1), dtype=np.float32)
    bv_s = np.ascontiguousarray(bv.reshape(DK, 1), dtype=np.float32)
    ones = np.ones((128, 1), dtype=np.float16)
    iden = np.eye(128, dtype=np.float16)
    onesf = np.ones((1, 128), dtype=np.float16)

    p = np.arange(128)[:, None]
    j = np.arange(64)[None, :]
    masks = []
    for h in (0, 1):
        m = np.zeros((128, 64), dtype=np.float32)
        m[(p < 64) & (p <= j)] = NEG
        if h == 1:
            m[p[:, 0] >= 64, :] = NEG
        masks.append(m)

    in_maps = []
    for core in range(NCORES):
        b, h = core // 2, core % 2
        xtc = inputs[b].T.reshape(D, 16, 2, 64)
        if h == 1:
            xtc = xtc[:, :, ::-1, :]
        xtc = np.ascontiguousarray(xtc).reshape(D, S).astype(np.float16)
        in_maps.append({
            "xt": xtc, "wq": wq_s, "wk": wk_s, "wv": wv_s,
            "bq": bq_s, "bk": bk_s, "bv": bv_s,
            "maskd": masks[h], "onesd": ones, "idend": iden,
            "onesfd": onesf,
        })
    return in_maps


def kernel(inputs, Wq, bq, Wk, bk, Wv, bv):
    inputs = np.asarray(inputs, dtype=np.float32)
    Wq, bq = np.asarray(Wq), np.asarray(bq)
    Wk, bk = np.asarray(Wk), np.asarray(bk)
    Wv, bv = np.asarray(Wv), np.asarray(bv)
    if "nc" not in _cache:
        _cache["nc"] = _build()
    nc = _cache["nc"]
    in_maps = _prep_inputs(inputs, Wq, bq, Wk, bk, Wv, bv)
    res = run_bass_kernel_spmd(nc, in_maps, list(range(NCORES)))
    out = np.empty((B, S, DK), dtype=np.float32)
    for core in range(NCORES):
        b, h = core // 2, core % 2
        oT = res.results[core]["outT"].astype(np.float32)  # [DK, 1024]
        o = oT.T.reshape(16, 64, DK)                       # [c, j, DK]
        out[b].reshape(16, 2, 64, DK)[:, h] = o
    # Row 2047 is fully masked: scores - 1e9 == -1e9 exactly in fp32, so
    # the reference's softmax over it is uniform -> mean(V). On device it
    # underflows to 0/0; patch it here.
    meanV = inputs.mean(axis=1) @ Wv + bv                  # [B, DK]
    out[:, S - 1, :] = meanV
    return out


# revision 44
# speedup vs baseline: 1.0094x; 1.0094x over previous
"""Masked self-attention Trainium2 kernel (8 NeuronCores, Bass/Tile).

Problem: B=4, S=2048, D=1024, DK=128 fp32.
  Q = X@Wq + bq; K = X@Wk + bk; V = X@Wv + bv
  scores = Q@K^T / sqrt(DK); masked = scores + tril(ones)*(-1e9)
  out = softmax(masked) @ V

Sharding: core = (batch b = core//2) x (row-half h = core%2). Each core
computes 64 query rows of each of the 16 query tiles of its batch
(rows 128c + 64h + j). All cores run an identical program; per-core
differences are carried entirely in the input data (a column
permutation of X^T and a small mask block).

Device layouts (all transposed so the PE contracts over partitions):
  X^T [D, S] (host-transposed, per-tile column permuted: own rows
  first), streamed in 2 superblocks of 1024 columns.
  Q^T/K^T/V^T [DK, *] = W-chunks(lhsT) x X^T(moving) fp16 matmuls with
  fp32 PSUM accumulation; Q projected only for the core's own 64-column
  tile halves (strided moving AP), 512 columns per superblock.
  scores^T [s-chunk 128, q-cols] = K^T-chunk(lhsT) x Q^T(moving)
  causal skip: chunk c only attends query tiles qi <= c -> contiguous
  q-prefix of width 64*(c+1); single [128,64] mask block on the
  diagonal tile. The -1e9 add absorbs the score entirely in fp32
  (|score| << ulp(1e9)), matching the reference bit pattern, and exp
  underflows masked lanes to exactly 0.
  softmax: exp without max-subtraction (scores are O(1)); row sums
  accumulated on the DVE into an fp16 staging tile (sacc += pt per
  piece) and reduced once per 512-column half with an M=1 all-ones
  matmul -- this removes a third matmul pass over every attention
  piece. Normalization via DVE reciprocal_approx_fast (no Ln/Exp
  activation-table reloads; the scalar engine only ever loads the Exp
  table) and a K=1 matmul broadcast of 1/sums across partitions.
  out^T [DK, 1024] accumulated in PSUM across s-chunks.

  Schedule: superblock-0 projections; then the attention pieces that
  need only chunks 7..1 (filling the PE while superblock 1 streams
  from HBM); superblock-1 projections; phase A (query columns
  [512,1024), chunks 15..8); phase B (columns [0,512), chunks 15..8);
  the chunk-0 piece last. The upper output half is summed/normalized/
  DMA'd while phase B still streams on the PE, so only the lower
  half's normalize chain sits on the critical tail. The globally
  fully-masked last row (2047) is patched on the host with mean(V) --
  exactly what the fp32 reference computes for it, since scores - 1e9
  == -1e9 in fp32 makes its softmax uniform. Its on-device column
  underflows to 0/0 = NaN and is overwritten.

  All matmul operands are float16 (11-bit mantissa, ~2.4e-4 rounding)
  with fp32 PSUM accumulation; the output is stored fp16 (halves the
  store DMA) and widened on the host. The first weight chunk gets a
  dedicated small first-wave DMA because the DGE queues fair-share HBM
  bandwidth and gate the first matmul.
"""

import numpy as np

import concourse.bacc as bacc
import concourse.tile as tile
import concourse.mybir as mybir
from concourse.bass_utils import run_bass_kernel_spmd

F32 = mybir.dt.float32
F16 = mybir.dt.float16
AF = mybir.ActivationFunctionType

B, S, D, DK = 4, 2048, 1024, 128
NEG = -1.0e9
NCORES = 8
NSB = 2           # superblocks of 1024 s-columns
NCHUNK = 16       # s-chunks of 128
QL = 1024         # local query columns per core (16 tiles x 64)

_cache = {}


def _build():
    nc = bacc.Bacc("TRN2", target_bir_lowering=False, debug=False,
                   num_devices=NCORES)

    xt = nc.dram_tensor("xt", [D, S], F16, kind="ExternalInput")
    wq = nc.dram_tensor("wq", [128, 8, DK], F16, kind="ExternalInput")
    wk = nc.dram_tensor("wk", [128, 8, DK], F16, kind="ExternalInput")
    wv = nc.dram_tensor("wv", [128, 8, DK], F16, kind="ExternalInput")
    bq = nc.dram_tensor("bq", [DK, 1], F32, kind="ExternalInput")
    bk = nc.dram_tensor("bk", [DK, 1], F32, kind="ExternalInput")
    bv = nc.dram_tensor("bv", [DK, 1], F32, kind="ExternalInput")
    maskd = nc.dram_tensor("maskd", [128, 64], F32, kind="ExternalInput")
    onesd = nc.dram_tensor("onesd", [128, 1], F16, kind="ExternalInput")
    idend = nc.dram_tensor("idend", [128, 128], F16, kind="ExternalInput")
    onesfd = nc.dram_tensor("onesfd", [1, 128], F16, kind="ExternalInput")
    outT = nc.dram_tensor("outT", [DK, QL], F16, kind="ExternalOutput")

    with tile.TileContext(nc) as tc:
        with (
            tc.tile_pool(name="consts", bufs=1) as cpool,
            tc.tile_pool(name="xblk", bufs=2) as xpool,
            tc.tile_pool(name="kv", bufs=1) as kvpool,
            tc.tile_pool(name="pt", bufs=6) as ppool,
            tc.tile_pool(name="outp", bufs=1) as opool,
            tc.tile_pool(name="ps_out", bufs=1, space="PSUM") as ps_out_pool,
            tc.tile_pool(name="ps_proj", bufs=2, space="PSUM") as ps_proj_pool,
            tc.tile_pool(name="ps_score", bufs=3, space="PSUM") as ps_score_pool,
            tc.tile_pool(name="ps_rb", bufs=1, space="PSUM") as ps_rb_pool,
        ):
            # ---- weights first (needed by the very first matmul).
            # wq's DMA is emitted after superblock 0's X so the K/V path
            # gets the HBM bandwidth first.
            w_sb = {}
            for name, dram in (("k", wk), ("v", wv), ("q", wq)):
                t = cpool.tile([128, 8, DK], F16, tag=f"w{name}")
                if name == "k":
                    nc.scalar.dma_start(out=t[:, 0:1], in_=dram[:, 0:1])
                    nc.scalar.dma_start(out=t[:, 1:8], in_=dram[:, 1:8])
                elif name == "v":
                    nc.scalar.dma_start(out=t[:], in_=dram[:])
                w_sb[name] = t

            def small_consts():
                b_sb = {}
                for name, dram in (("q", bq), ("k", bk), ("v", bv)):
                    t = cpool.tile([DK, 1], F32, tag=f"b{name}")
                    nc.gpsimd.dma_start(out=t[:], in_=dram[:])
                    b_sb[name] = t
                mask_sb = cpool.tile([128, 64], F32, tag="mask")
                nc.gpsimd.dma_start(out=mask_sb[:], in_=maskd[:])
                ones_sb = cpool.tile([128, 1], F16, tag="ones")
                nc.gpsimd.dma_start(out=ones_sb[:], in_=onesd[:])
                iden_sb = cpool.tile([128, 128], F16, tag="iden")
                nc.gpsimd.dma_start(out=iden_sb[:], in_=idend[:])
                onesf_sb = cpool.tile([1, 128], F16, tag="onesf")
                nc.gpsimd.dma_start(out=onesf_sb[:], in_=onesfd[:])
                return b_sb, mask_sb, ones_sb, iden_sb, onesf_sb

            # ---- persistent buffers ----
            kT_sb = kvpool.tile([DK, S], F16, tag="kT")
            vT_sb = kvpool.tile([DK, S], F16, tag="vT")
            qT_sb = kvpool.tile([DK, QL], F16, tag="qT")
            vnat_sb = kvpool.tile([128, NCHUNK, DK], F16, tag="vnat")
            sacc_sb = kvpool.tile([128, QL], F16, tag="sacc")
            o_sb = opool.tile([DK, QL], F16, tag="o")
            recip_u = opool.tile([1, 512], F32, tag="recip_u")
            recip_l = opool.tile([1, 512], F32, tag="recip_l")
            recip_u16 = opool.tile([1, 512], F16, tag="recip_u16")
            recip_l16 = opool.tile([1, 512], F16, tag="recip_l16")
            rbu_sb = opool.tile([128, 512], F32, tag="rbu")
            rbl_sb = opool.tile([128, 512], F32, tag="rbl")

            ps_out = ps_out_pool.tile([DK, QL], F32)       # 2 banks
            nc.vector.memset(ps_out[:], 0.0)
            nc.vector.memset(sacc_sb[:], 0.0)

            # ---- projections for one superblock of 1024 columns ----
            def proj_superblock(sb, b_sb, iden_sb):
                s0 = 1024 * sb
                xb = xpool.tile([128, 8, 1024], F16, tag="xb")
                # sb0 on the sync queue, sb1 on the scalar queue so both
                # issue early and share HBM fairly.
                eng = nc.sync
                # Fine-grained pieces let the dc-accumulation matmuls start
                # on partial data instead of stalling for a full 1 MiB half.
                for (d0, d1, half) in ((0, 2, 0), (2, 5, 0), (5, 8, 0),
                                       (0, 4, 1), (4, 8, 1)):
                    eng.dma_start(
                        out=xb[:, d0:d1, 512 * half:512 * half + 512],
                        in_=xt[128 * d0:128 * d1,
                               s0 + 512 * half:s0 + 512 * half + 512]
                        .rearrange("(i p) s -> p i s", p=128))
                if sb == 0:
                    # wq (not needed until Q proj ~19us in) goes on the
                    # gpsimd queue, out of the early HBM rush
                    nc.gpsimd.dma_start(out=w_sb["q"][:], in_=wq[:])

                for half in range(2):
                    cl = slice(s0 + 512 * half, s0 + 512 * half + 512)
                    xl = slice(512 * half, 512 * half + 512)
                    for name, dst in (("k", kT_sb), ("v", vT_sb)):
                        pp = ps_proj_pool.tile([DK, 512], F32, tag="pp")
                        for dc in range(8):
                            nc.tensor.matmul(
                                pp[:], w_sb[name][:, dc], xb[:, dc, xl],
                                start=(dc == 0), stop=(dc == 7))
                        nc.scalar.activation(dst[:, cl], pp[:],
                                             AF.Identity,
                                             bias=b_sb[name][:])

                # Q: first 64 cols of each 128-tile (own queries)
                pq = ps_proj_pool.tile([DK, 512], F32, tag="pp")
                for dc in range(8):
                    qmov = (xb[:, dc].rearrange("p (t j) -> p t j", t=8)
                            [:, :, 0:64])
                    nc.tensor.matmul(pq[:], w_sb["q"][:, dc], qmov,
                                     start=(dc == 0), stop=(dc == 7))
                q0 = 512 * sb
                nc.scalar.activation(qT_sb[:, q0:q0 + 512], pq[:],
                                     AF.Identity, bias=b_sb["q"][:])

                # V natural tiles (transpose V^T chunks)
                for g in range(2):
                    tp = ps_proj_pool.tile([128, 4, 128], F16, tag="pp")
                    for t in range(4):
                        c = 8 * sb + 4 * g + t
                        nc.tensor.matmul(
                            tp[:, t], vT_sb[:, 128 * c:128 * c + 128],
                            iden_sb[:], is_transpose=True,
                            start=(t == 0), stop=(t == 3))
                    c0 = 8 * sb + 4 * g
                    nc.vector.tensor_copy(vnat_sb[:, c0:c0 + 4], tp[:])

            # ---- attention pieces, software-pipelined by one stage:
            # piece i's PV matmul is emitted after piece i+1's score+exp,
            # so the PE always has a score to run while an exp is pending
            pending = []

            def _flush_one():
                c, p0, pn, pt, stop = pending.pop(0)
                nc.tensor.matmul(ps_out[:, p0:p0 + pn], vnat_sb[:, c],
                                 pt[:, 0:pn], start=False, stop=stop)
                nc.vector.tensor_tensor(
                    sacc_sb[:, p0:p0 + pn], sacc_sb[:, p0:p0 + pn],
                    pt[:, 0:pn], mybir.AluOpType.add)

            def flush_all():
                while pending:
                    _flush_one()

            def attn_piece(c, p0, pn, stop=False):
                kT_c = kT_sb[:, 128 * c:128 * c + 128]
                sc = ps_score_pool.tile([128, 512], F32, tag="sc")
                nc.tensor.matmul(sc[:, 0:pn], kT_c, qT_sb[:, p0:p0 + pn],
                                 start=True, stop=True)
                dcol = 64 * c
                if p0 <= dcol < p0 + pn:
                    dl = dcol - p0
                    nc.vector.tensor_tensor(
                        sc[:, dl:dl + 64], sc[:, dl:dl + 64],
                        mask_sb[:], mybir.AluOpType.add)
                pt = ppool.tile([128, 512], F16, tag="pt")
                nc.scalar.activation(pt[:, 0:pn], sc[:, 0:pn], AF.Exp)
                while len(pending) >= 2:
                    _flush_one()
                pending.append((c, p0, pn, pt, stop))

            # --- schedule ---
            # superblock 0 projections
            # (consts are DMA'd on the gpsimd queue concurrently)
            (b_sb, mask_sb, ones_sb, iden_sb, onesf_sb) = small_consts()
            proj_superblock(0, b_sb, iden_sb)

            # early pieces: chunks 7..1 (chunk 0 is saved for the very end)
            for c in range(7, 0, -1):
                attn_piece(c, 0, 64 * (c + 1))

            # superblock 1 projections
            proj_superblock(1, b_sb, iden_sb)

            # Phase A: query columns [512, 1024) -- chunks 15..8
            for c in range(15, 7, -1):
                attn_piece(c, 512, 64 * (c + 1) - 512, stop=(c == 8))

            # Phase B: chunks 15..8, columns [0, 512)
            sums_u = None
            for c in range(15, 7, -1):
                attn_piece(c, 0, 512)
                if c == 15:
                    # upper half row sums: needs phase A's sacc adds, which
                    # this flush completes without draining the exp pipeline
                    flush_all()
                    sums_u = ps_rb_pool.tile([1, 512], F32, tag="rbsum")
                    nc.tensor.matmul(sums_u[:], ones_sb[:, 0:1],
                                     sacc_sb[:, 512:1024],
                                     start=True, stop=True)
                if c == 12:
                    # upper finalize, overlapping the remaining pieces
                    nc.vector.reciprocal_approx_fast(recip_u[:], sums_u[:])
                    nc.vector.tensor_copy(recip_u16[:], recip_u[:])
                    rb_u = ps_rb_pool.tile([128, 512], F32, tag="rbsum")
                    nc.tensor.matmul(rb_u[:], onesf_sb[:], recip_u16[:],
                                     start=True, stop=True)
                if c == 10:
                    nc.vector.tensor_copy(rbu_sb[:], rb_u[:])
                    nc.vector.tensor_tensor(
                        o_sb[:, 512:1024], ps_out[:, 512:1024],
                        rbu_sb[:], mybir.AluOpType.mult)
                    nc.gpsimd.dma_start(out=outT[:, 512:1024],
                                        in_=o_sb[:, 512:1024])

            # final tiny piece: chunk 0, columns [0, 64); flush pending
            # PV/sum work first so only chunk 0's remains on the tail
            flush_all()
            attn_piece(0, 0, 64, stop=True)

            # lower finalize
            flush_all()
            sums_l = ps_rb_pool.tile([1, 512], F32, tag="rbsum")
            nc.tensor.matmul(sums_l[:], ones_sb[:, 0:1], sacc_sb[:, 0:512],
                             start=True, stop=True)
            nc.vector.reciprocal_approx_fast(recip_l[:], sums_l[:])
            nc.vector.tensor_copy(recip_l16[:], recip_l[:])
            rb_l = ps_rb_pool.tile([128, 512], F32, tag="rbsum")
            nc.tensor.matmul(rb_l[:], onesf_sb[:], recip_l16[:],
                             start=True, stop=True)
            nc.vector.tensor_copy(rbl_sb[:], rb_l[:])
            nc.vector.tensor_tensor(o_sb[:, 0:512], ps_out[:, 0:512],
                                    rbl_sb[:], mybir.AluOpType.mult)
            nc.sync.dma_start(out=outT[:, 0:512], in_=o_sb[:, 0:512])

    nc.compile()
    return nc


def _prep_inputs(inputs, Wq, bq, Wk, bk, Wv, bv):
    scale = np.float32(1.0 / np.sqrt(DK))
    wq_s = np.ascontiguousarray((Wq * scale).reshape(8, 128, DK).transpose(1, 0, 2)).astype(np.float16)
    wk_s = np.ascontiguousarray(Wk.reshape(8, 128, DK).transpose(1, 0, 2)).astype(np.float16)
    wv_s = np.ascontiguousarray(Wv.reshape(8, 128, DK).transpose(1, 0, 2)).astype(np.float16)
    bq_s = np.ascontiguousarray((bq * scale).reshape(DK, 1), dtype=np.float32)
    bk_s = np.ascontiguousarray(bk.reshape(DK, 1), dtype=np.float32)
    bv_s = np.ascontiguousarray(bv.reshape(DK, 1), dtype=np.float32)
    ones = np.ones((128, 1), dtype=np.float16)
    iden = np.eye(128, dtype=np.float16)
    onesf = np.ones((1, 128), dtype=np.float16)

    p = np.arange(128)[:, None]
    j = np.arange(64)[None, :]
    masks = []
    for h in (0, 1):
        m = np.zeros((128, 64), dtype=np.float32)
        m[(p < 64) & (p <= j)] = NEG
        if h == 1:
            m[p[:, 0] >= 64, :] = NEG
        masks.append(m)

    in_maps = []
    for core in range(NCORES):
        b, h = core // 2, core % 2
        xtc = inputs[b].T.reshape(D, 16, 2, 64)
        if h == 1:
            xtc = xtc[:, :, ::-1, :]
        xtc = np.ascontiguousarray(xtc).reshape(D, S).astype(np.float16)
        in_maps.append({
            "xt": xtc, "wq": wq_s, "wk": wk_s, "wv": wv_s,
            "bq": bq_s, "bk": bk_s, "bv": bv_s,
            "maskd": masks[h], "onesd": ones, "idend": iden,
            "onesfd": onesf,
        })
    return in_maps


def kernel(inputs, Wq, bq, Wk, bk, Wv, bv):
    inputs = np.asarray(inputs, dtype=np.float32)
    Wq, bq = np.asarray(Wq), np.asarray(bq)
    Wk, bk = np.asarray(Wk), np.asarray(bk)
    Wv, bv = np.asarray(Wv), np.asarray(bv)
    if "nc" not in _cache:
        _cache["nc"] = _build()
    nc = _cache["nc"]
    in_maps = _prep_inputs(inputs, Wq, bq, Wk, bk, Wv, bv)
    res = run_bass_kernel_spmd(nc, in_maps, list(range(NCORES)))
    out = np.empty((B, S, DK), dtype=np.float32)
    for core in range(NCORES):
        b, h = core // 2, core % 2
        oT = res.results[core]["outT"].astype(np.float32)  # [DK, 1024]
        o = oT.T.reshape(16, 64, DK)                       # [c, j, DK]
        out[b].reshape(16, 2, 64, DK)[:, h] = o
    # Row 2047 is fully masked: scores - 1e9 == -1e9 exactly in fp32, so
    # the reference's softmax over it is uniform -> mean(V). On device it
    # underflows to 0/0; patch it here.
    meanV = inputs.mean(axis=1) @ Wv + bv                  # [B, DK]
    out[:, S - 1, :] = meanV
    return out
